# revision 8
# baseline (speedup 1.0000x reference)
"""v6: 4-image 2x2 PE tile grid, K=64 contraction, no delta copies.

Four matmul streams run concurrently on the PE (measured: 4 tiles in a
2x2 grid of (64-row, 64-col) positions each stream at ~1 px/cycle):
  img0: data parts 0-63,   tile (0,0)   -> psumA[0:64]
  img1: data parts 0-63,   tile (0,64)  -> psumA[64:128]
  img2: data parts 64-127, tile (64,0)  -> psumB[0:64]
  img3: data parts 64-127, tile (64,64) -> psumB[64:128]
All 25 taps accumulate into each image's own psum region; dy/dx shifts
come free via rhs slicing (no host-side shifted copies -> half the DMA
of the delta-packed variant). Per 8-row block: 100 matmuls of F=864 at
4-way concurrency ~= 21.6k PE cycles.
"""

import numpy as np
import ml_dtypes

B, C, H, W = 32, 64, 112, 112
O, K, KS = 64, 8, 5
HO, WO = H - KS + 1, W - KS + 1
N_CORES = 8
NTAP = KS * KS
RB = 4                      # output rows per block (F=432 <= 512 psum bank limit)
N_BLOCKS = (HO + RB - 1) // RB   # 13 full + 1 tail of 4

_built = None


def _build_nc():
    import concourse.tile as tile
    import concourse.mybir as mybir
    from concourse import bacc

    nc = bacc.Bacc(None)
    # x[half][img-in-half]: half 0 -> partitions 0-63 (img0, img1),
    # half 1 -> partitions 64-127 (img2, img3)
    x = nc.dram_tensor("x", [2, 2, 64, H, W], mybir.dt.bfloat16,
                       kind="ExternalInput")
    w = nc.dram_tensor("w", [128, NTAP * 64], mybir.dt.bfloat16,
                       kind="ExternalInput")
    bt = nc.dram_tensor("b", [128, 1], mybir.dt.float32, kind="ExternalInput")
    # y[pair]: pair 0 = (img0, img1) from psumA, pair 1 = (img2, img3)
    y = nc.dram_tensor("y", [2, 128, HO * WO], mybir.dt.float32,
                       kind="ExternalOutput")

    with tile.TileContext(nc) as tc:
        with (
            tc.tile_pool(name="wp", bufs=1) as wp,
            tc.tile_pool(name="xp", bufs=2) as xp,
            tc.tile_pool(name="op", bufs=4) as op,
            tc.tile_pool(name="bp", bufs=1) as bp,
            tc.tile_pool(name="ps", bufs=3, space="PSUM") as ps,
        ):
            warm = wp.tile([128, 64], mybir.dt.bfloat16, tag="warm")
            nc.vector.memset(warm[:], 0.0)

            w3 = w.rearrange("p (t m) -> p t m", t=NTAP)
            wt = wp.tile([128, NTAP, 64], mybir.dt.bfloat16)

            # 4 image tiles: [64, H, W] each; imgs 0,1 on parts 0-63,
            # imgs 2,3 on parts 64-127
            xt = [xp.tile([128, H, W], mybir.dt.bfloat16, tag="xt",
                          name=f"xt{i}") for i in range(2)]

            # first rows of all four images (split finely across queues),
            # then weights, then bands
            FIRST = 8
            for i in range(2):
                for hh in range(2):
                    for rr in range(0, FIRST, 4):
                        nc.sync.dma_start(
                            xt[i][64 * hh:64 * hh + 64, rr:rr + 4, :],
                            x[hh, i][:, rr:rr + 4, :])
            for t0, t1 in ((0, 5), (5, 10), (10, 15), (15, 20), (20, NTAP)):
                nc.sync.dma_start(wt[:, t0:t1, :], w3[:, t0:t1, :])
            bias = bp.tile([128, 1], mybir.dt.float32)
            nc.sync.dma_start(bias[:], bt[:])

            # p-state warmup while DMA fills
            wpsum = ps.tile([128, RB * WO], mybir.dt.float32, tag="pa")
            for i in range(64):
                nc.tensor.matmul(wpsum[:64, :64], warm[:], warm[:],
                                 start=True, stop=True)
            BAND = 20
            for b0 in range(FIRST, H, BAND):
                b1 = min(b0 + BAND, H)
                for i in range(2):
                    for hh in range(2):
                        nc.sync.dma_start(
                            xt[i][64 * hh:64 * hh + 64, b0:b1, :],
                            x[hh, i][:, b0:b1, :])

            for blk in range(N_BLOCKS):
                y0 = blk * RB
                rb = min(RB, HO - y0)
                ff = rb * WO
                pa = ps.tile([128, RB * WO], mybir.dt.float32, tag="pa")
                pb = ps.tile([128, RB * WO], mybir.dt.float32, tag="pb")
                for t in range(NTAP):
                    dy, dx = divmod(t, KS)
                    first, last = (t == 0), (t == NTAP - 1)
                    for j in range(4):
                        hh, col = j // 2, j % 2
                        pdst = pa if hh == 0 else pb
                        nc.tensor.matmul(
                            pdst[64 * col:64 * col + 64, :ff],
                            wt[64 * hh:64 * hh + 64, t, :],
                            xt[col][64 * hh:64 * hh + 64,
                                    y0 + dy:y0 + dy + rb,
                                    dx:dx + WO],
                            start=first, stop=last,
                            tile_position=(64 * hh, 64 * col),
                        )
                oa = op.tile([128, RB * WO], mybir.dt.float32)
                ob = op.tile([128, RB * WO], mybir.dt.float32)
                nc.vector.tensor_scalar_add(oa[:, :ff], pa[:, :ff], bias[:])
                nc.vector.tensor_scalar_add(ob[:, :ff], pb[:, :ff], bias[:])
                nc.sync.dma_start(y[0][:, y0 * WO:(y0 + rb) * WO], oa[:, :ff])
                nc.sync.dma_start(y[1][:, y0 * WO:(y0 + rb) * WO], ob[:, :ff])
    nc.finalize()
    return nc


def _prep_inputs(X, weight, bias, sel):
    weight = np.asarray(weight)
    sel = np.asarray(sel)
    w64 = np.zeros((KS, KS, C, O), dtype=np.float32)
    wflat = weight.astype(np.float32)
    for o in range(O):
        for j in range(K):
            w64[:, :, int(sel[o, j]), o] += wflat[o, j]
    # taps laid out t = dy*5+dx; weight rows duplicated into both halves
    w2 = np.zeros((128, NTAP, O), dtype=np.float32)
    for dy in range(KS):
        for dx in range(KS):
            w2[0:C, dy * KS + dx, :] = w64[dy, dx]
    w2[C:2 * C] = w2[0:C]
    w_host = np.ascontiguousarray(w2.reshape(128, NTAP * O)).astype(
        ml_dtypes.bfloat16)

    b_host = np.tile(np.asarray(bias, dtype=np.float32), 2).reshape(128, 1)

    xb = np.asarray(X, dtype=np.float32).astype(ml_dtypes.bfloat16)
    # core i handles images 4i..4i+3: x[hh, j] = image 4i + 2*hh + j
    # (hh selects partition half, j selects which xt tile / psum col half)
    xcores = xb.reshape(N_CORES, 2, 2, C, H, W)

    in_maps = [
        {"x": np.ascontiguousarray(xcores[i]), "w": w_host, "b": b_host}
        for i in range(N_CORES)
    ]
    return in_maps


def _postprocess(results):
    outs = []
    for r in results:
        # y[hh, colhalf]: psum pair hh holds image 2*hh + colhalf
        outs.append(r["y"].reshape(4, O, HO, WO))
    out = np.concatenate(outs, axis=0).astype(np.float32)
    return out


def kernel(X, weight, bias, sel):
    global _built
    from concourse.bass_utils import run_bass_kernel_spmd

    assert X.shape == (B, C, H, W), X.shape
    if _built is None:
        _built = _build_nc()
    in_maps = _prep_inputs(X, weight, bias, sel)
    res = run_bass_kernel_spmd(
        _built, in_maps, core_ids=list(range(N_CORES)), trace=False
    )
    return _postprocess(res.results)


# revision 9
# speedup vs baseline: 1.0696x; 1.0696x over previous
"""v6: 4-image 2x2 PE tile grid, K=64 contraction, no delta copies.

Four matmul streams run concurrently on the PE (measured: 4 tiles in a
2x2 grid of (64-row, 64-col) positions each stream at ~1 px/cycle):
  img0: data parts 0-63,   tile (0,0)   -> psumA[0:64]
  img1: data parts 0-63,   tile (0,64)  -> psumA[64:128]
  img2: data parts 64-127, tile (64,0)  -> psumB[0:64]
  img3: data parts 64-127, tile (64,64) -> psumB[64:128]
All 25 taps accumulate into each image's own psum region; dy/dx shifts
come free via rhs slicing (no host-side shifted copies -> half the DMA
of the delta-packed variant). Per 8-row block: 100 matmuls of F=864 at
4-way concurrency ~= 21.6k PE cycles.
"""

import numpy as np
import ml_dtypes

B, C, H, W = 32, 64, 112, 112
O, K, KS = 64, 8, 5
HO, WO = H - KS + 1, W - KS + 1
N_CORES = 8
NTAP = KS * KS
RB = 4                      # output rows per block (F=432 <= 512 psum bank limit)
N_BLOCKS = (HO + RB - 1) // RB   # 13 full + 1 tail of 4

_built = None


def _build_nc():
    import concourse.tile as tile
    import concourse.mybir as mybir
    from concourse import bacc

    nc = bacc.Bacc(None)
    # x[p, i, h, w]: partition p = hh*64 + channel (hh = image pair),
    # i = image-in-pair (psum col half)
    x = nc.dram_tensor("x", [128, 2, H, W], mybir.dt.bfloat16,
                       kind="ExternalInput")
    w = nc.dram_tensor("w", [128, NTAP * 64], mybir.dt.bfloat16,
                       kind="ExternalInput")
    bt = nc.dram_tensor("b", [128, 1], mybir.dt.float32, kind="ExternalInput")
    # y[pair]: pair 0 = (img0, img1) from psumA, pair 1 = (img2, img3)
    y = nc.dram_tensor("y", [2, 128, HO * WO], mybir.dt.float32,
                       kind="ExternalOutput")

    with tile.TileContext(nc) as tc:
        with (
            tc.tile_pool(name="wp", bufs=1) as wp,
            tc.tile_pool(name="xp", bufs=1) as xp,
            tc.tile_pool(name="op", bufs=4) as op,
            tc.tile_pool(name="bp", bufs=1) as bp,
            tc.tile_pool(name="ps", bufs=3, space="PSUM") as ps,
        ):
            warm = wp.tile([128, 64], mybir.dt.bfloat16, tag="warm")
            nc.vector.memset(warm[:], 0.0)

            w3 = w.rearrange("p (t m) -> p t m", t=NTAP)
            wt = wp.tile([128, NTAP, 64], mybir.dt.bfloat16)

            # all 4 images in one tile: [128, 2, H, W]
            xt = xp.tile([128, 2, H, W], mybir.dt.bfloat16, tag="xt")

            # first rows of all four images in ONE call (one DIRECT2D),
            # then weights, then bands
            FIRST = 8
            nc.sync.dma_start(xt[:, :, 0:FIRST, :], x[:, :, 0:FIRST, :])
            for t0, t1 in ((0, 13), (13, NTAP)):
                nc.sync.dma_start(wt[:, t0:t1, :], w3[:, t0:t1, :])
            bias = bp.tile([128, 1], mybir.dt.float32)
            nc.sync.dma_start(bias[:], bt[:])

            # p-state warmup while DMA fills
            wpsum = ps.tile([128, RB * WO], mybir.dt.float32, tag="pa")
            for i in range(64):
                nc.tensor.matmul(wpsum[:64, :64], warm[:], warm[:],
                                 start=True, stop=True)
            BAND = 16
            for b0 in range(FIRST, H, BAND):
                b1 = min(b0 + BAND, H)
                nc.sync.dma_start(xt[:, :, b0:b1, :], x[:, :, b0:b1, :])

            for blk in range(N_BLOCKS):
                y0 = blk * RB
                rb = min(RB, HO - y0)
                ff = rb * WO
                pa = ps.tile([128, RB * WO], mybir.dt.float32, tag="pa")
                pb = ps.tile([128, RB * WO], mybir.dt.float32, tag="pb")
                for t in range(NTAP):
                    dy, dx = divmod(t, KS)
                    first, last = (t == 0), (t == NTAP - 1)
                    for j in range(4):
                        hh, col = j // 2, j % 2
                        pdst = pa if hh == 0 else pb
                        nc.tensor.matmul(
                            pdst[64 * col:64 * col + 64, :ff],
                            wt[64 * hh:64 * hh + 64, t, :],
                            xt[64 * hh:64 * hh + 64, col,
                               y0 + dy:y0 + dy + rb,
                               dx:dx + WO],
                            start=first, stop=last,
                            tile_position=(64 * hh, 64 * col),
                        )
                oa = op.tile([128, RB * WO], mybir.dt.float32)
                ob = op.tile([128, RB * WO], mybir.dt.float32)
                nc.vector.tensor_scalar_add(oa[:, :ff], pa[:, :ff], bias[:])
                nc.vector.tensor_scalar_add(ob[:, :ff], pb[:, :ff], bias[:])
                nc.sync.dma_start(y[0][:, y0 * WO:(y0 + rb) * WO], oa[:, :ff])
                nc.sync.dma_start(y[1][:, y0 * WO:(y0 + rb) * WO], ob[:, :ff])
    nc.finalize()
    return nc


def _prep_inputs(X, weight, bias, sel):
    weight = np.asarray(weight)
    sel = np.asarray(sel)
    w64 = np.zeros((KS, KS, C, O), dtype=np.float32)
    wflat = weight.astype(np.float32)
    for o in range(O):
        for j in range(K):
            w64[:, :, int(sel[o, j]), o] += wflat[o, j]
    # taps laid out t = dy*5+dx; weight rows duplicated into both halves
    w2 = np.zeros((128, NTAP, O), dtype=np.float32)
    for dy in range(KS):
        for dx in range(KS):
            w2[0:C, dy * KS + dx, :] = w64[dy, dx]
    w2[C:2 * C] = w2[0:C]
    w_host = np.ascontiguousarray(w2.reshape(128, NTAP * O)).astype(
        ml_dtypes.bfloat16)

    b_host = np.tile(np.asarray(bias, dtype=np.float32), 2).reshape(128, 1)

    xb = np.asarray(X, dtype=np.float32).astype(ml_dtypes.bfloat16)
    # core n handles images 4n..4n+3: partition hh*64+c holds channel c of
    # image pair hh; free index i selects image 4n + 2*hh + i
    xcores = xb.reshape(N_CORES, 2, 2, C, H, W).transpose(0, 1, 3, 2, 4, 5)
    xcores = xcores.reshape(N_CORES, 128, 2, H, W)

    in_maps = [
        {"x": np.ascontiguousarray(xcores[i]), "w": w_host, "b": b_host}
        for i in range(N_CORES)
    ]
    return in_maps


def _postprocess(results):
    outs = []
    for r in results:
        # y[hh, colhalf]: psum pair hh holds image 2*hh + colhalf
        outs.append(r["y"].reshape(4, O, HO, WO))
    out = np.concatenate(outs, axis=0).astype(np.float32)
    return out


def kernel(X, weight, bias, sel):
    global _built
    from concourse.bass_utils import run_bass_kernel_spmd

    assert X.shape == (B, C, H, W), X.shape
    if _built is None:
        _built = _build_nc()
    in_maps = _prep_inputs(X, weight, bias, sel)
    res = run_bass_kernel_spmd(
        _built, in_maps, core_ids=list(range(N_CORES)), trace=False
    )
    return _postprocess(res.results)


# revision 10
# speedup vs baseline: 1.0781x; 1.0080x over previous
"""v6: 4-image 2x2 PE tile grid, K=64 contraction, no delta copies.

Four matmul streams run concurrently on the PE (measured: 4 tiles in a
2x2 grid of (64-row, 64-col) positions each stream at ~1 px/cycle):
  img0: data parts 0-63,   tile (0,0)   -> psumA[0:64]
  img1: data parts 0-63,   tile (0,64)  -> psumA[64:128]
  img2: data parts 64-127, tile (64,0)  -> psumB[0:64]
  img3: data parts 64-127, tile (64,64) -> psumB[64:128]
All 25 taps accumulate into each image's own psum region; dy/dx shifts
come free via rhs slicing (no host-side shifted copies -> half the DMA
of the delta-packed variant). Per 8-row block: 100 matmuls of F=864 at
4-way concurrency ~= 21.6k PE cycles.
"""

import numpy as np
import ml_dtypes

B, C, H, W = 32, 64, 112, 112
O, K, KS = 64, 8, 5
HO, WO = H - KS + 1, W - KS + 1
N_CORES = 8
NTAP = KS * KS
RB = 4                      # output rows per block (F=432 <= 512 psum bank limit)
N_BLOCKS = (HO + RB - 1) // RB   # 13 full + 1 tail of 4

_built = None


def _build_nc():
    import concourse.tile as tile
    import concourse.mybir as mybir
    from concourse import bacc

    nc = bacc.Bacc(None)
    # x[p, i, h, w]: partition p = hh*64 + channel (hh = image pair),
    # i = image-in-pair (psum col half)
    x = nc.dram_tensor("x", [128, 2, H, W], mybir.dt.bfloat16,
                       kind="ExternalInput")
    w = nc.dram_tensor("w", [128, NTAP * 64], mybir.dt.bfloat16,
                       kind="ExternalInput")
    bt = nc.dram_tensor("b", [128, 1], mybir.dt.float32, kind="ExternalInput")
    # y[pair]: pair 0 = (img0, img1) from psumA, pair 1 = (img2, img3)
    y = nc.dram_tensor("y", [2, 128, HO * WO], mybir.dt.float32,
                       kind="ExternalOutput")

    with tile.TileContext(nc) as tc:
        with (
            tc.tile_pool(name="wp", bufs=1) as wp,
            tc.tile_pool(name="xp", bufs=1) as xp,
            tc.tile_pool(name="op", bufs=4) as op,
            tc.tile_pool(name="bp", bufs=1) as bp,
            tc.tile_pool(name="ps", bufs=3, space="PSUM") as ps,
        ):
            warm = wp.tile([128, 64], mybir.dt.bfloat16, tag="warm")
            nc.vector.memset(warm[:], 0.0)

            w3 = w.rearrange("p (t m) -> p t m", t=NTAP)
            wt = wp.tile([128, NTAP, 64], mybir.dt.bfloat16)

            # all 4 images in one tile: [128, 2, H, W]
            xt = xp.tile([128, 2, H, W], mybir.dt.bfloat16, tag="xt")

            # first rows of all four images in ONE call (one DIRECT2D),
            # then weights, then bands
            FIRST = 8
            nc.sync.dma_start(xt[:, :, 0:4, :], x[:, :, 0:4, :])
            nc.sync.dma_start(xt[:, :, 4:FIRST, :], x[:, :, 4:FIRST, :])
            for t0, t1 in ((0, 13), (13, NTAP)):
                nc.sync.dma_start(wt[:, t0:t1, :], w3[:, t0:t1, :])
            bias = bp.tile([128, 1], mybir.dt.float32)
            nc.sync.dma_start(bias[:], bt[:])

            # p-state warmup while DMA fills
            wpsum = ps.tile([128, RB * WO], mybir.dt.float32, tag="pa")
            for i in range(64):
                nc.tensor.matmul(wpsum[:64, :64], warm[:], warm[:],
                                 start=True, stop=True)
            BAND = 16
            for b0 in range(FIRST, H, BAND):
                b1 = min(b0 + BAND, H)
                nc.sync.dma_start(xt[:, :, b0:b1, :], x[:, :, b0:b1, :])

            for blk in range(N_BLOCKS):
                y0 = blk * RB
                rb = min(RB, HO - y0)
                ff = rb * WO
                pa = ps.tile([128, RB * WO], mybir.dt.float32, tag="pa")
                pb = ps.tile([128, RB * WO], mybir.dt.float32, tag="pb")
                for t in range(NTAP):
                    dy, dx = divmod(t, KS)
                    first, last = (t == 0), (t == NTAP - 1)
                    for j in range(4):
                        hh, col = j // 2, j % 2
                        pdst = pa if hh == 0 else pb
                        nc.tensor.matmul(
                            pdst[64 * col:64 * col + 64, :ff],
                            wt[64 * hh:64 * hh + 64, t, :],
                            xt[64 * hh:64 * hh + 64, col,
                               y0 + dy:y0 + dy + rb,
                               dx:dx + WO],
                            start=first, stop=last,
                            tile_position=(64 * hh, 64 * col),
                        )
                oa = op.tile([128, RB * WO], mybir.dt.float32)
                ob = op.tile([128, RB * WO], mybir.dt.float32)
                nc.vector.tensor_scalar_add(oa[:, :ff], pa[:, :ff], bias[:])
                nc.vector.tensor_scalar_add(ob[:, :ff], pb[:, :ff], bias[:])
                nc.sync.dma_start(y[0][:, y0 * WO:(y0 + rb) * WO], oa[:, :ff])
                nc.sync.dma_start(y[1][:, y0 * WO:(y0 + rb) * WO], ob[:, :ff])
    nc.finalize()
    return nc


def _prep_inputs(X, weight, bias, sel):
    weight = np.asarray(weight)
    sel = np.asarray(sel)
    w64 = np.zeros((KS, KS, C, O), dtype=np.float32)
    wflat = weight.astype(np.float32)
    for o in range(O):
        for j in range(K):
            w64[:, :, int(sel[o, j]), o] += wflat[o, j]
    # taps laid out t = dy*5+dx; weight rows duplicated into both halves
    w2 = np.zeros((128, NTAP, O), dtype=np.float32)
    for dy in range(KS):
        for dx in range(KS):
            w2[0:C, dy * KS + dx, :] = w64[dy, dx]
    w2[C:2 * C] = w2[0:C]
    w_host = np.ascontiguousarray(w2.reshape(128, NTAP * O)).astype(
        ml_dtypes.bfloat16)

    b_host = np.tile(np.asarray(bias, dtype=np.float32), 2).reshape(128, 1)

    xb = np.asarray(X, dtype=np.float32).astype(ml_dtypes.bfloat16)
    # core n handles images 4n..4n+3: partition hh*64+c holds channel c of
    # image pair hh; free index i selects image 4n + 2*hh + i
    xcores = xb.reshape(N_CORES, 2, 2, C, H, W).transpose(0, 1, 3, 2, 4, 5)
    xcores = xcores.reshape(N_CORES, 128, 2, H, W)

    in_maps = [
        {"x": np.ascontiguousarray(xcores[i]), "w": w_host, "b": b_host}
        for i in range(N_CORES)
    ]
    return in_maps


def _postprocess(results):
    outs = []
    for r in results:
        # y[hh, colhalf]: psum pair hh holds image 2*hh + colhalf
        outs.append(r["y"].reshape(4, O, HO, WO))
    out = np.concatenate(outs, axis=0).astype(np.float32)
    return out


def kernel(X, weight, bias, sel):
    global _built
    from concourse.bass_utils import run_bass_kernel_spmd

    assert X.shape == (B, C, H, W), X.shape
    if _built is None:
        _built = _build_nc()
    in_maps = _prep_inputs(X, weight, bias, sel)
    res = run_bass_kernel_spmd(
        _built, in_maps, core_ids=list(range(N_CORES)), trace=False
    )
    return _postprocess(res.results)
